# revision 18
# baseline (speedup 1.0000x reference)
"""Trainium2 Bass kernel for nn_EncoderBlock (visual/topic attention encoder + vocab MLP).

Sharding: 8 cores = 2 batch-groups x 4 vocab-groups.
Each core computes the attention front-end for its 8 batches (+ a replicated
global-batch-0 "sentinel" pass appended as a 9th batch block, needed for the
gama gate), entirely in transposed [E, tokens] layout, then projects its
vocab slice (7500 cols) of the MLP.  Host does layout prep + unshard only.
"""

import numpy as np

import concourse.bass as bass
import concourse.mybir as mybir
import concourse.tile as tile
from concourse import bacc
from concourse.bass_utils import run_bass_kernel_spmd
from concourse.masks import make_identity
from contextlib import ExitStack

F32 = mybir.dt.float32
AF = mybir.ActivationFunctionType
ALU = mybir.AluOpType
AX = mybir.AxisListType

EMBED = HIDDEN = 512
VOCAB = 30000
B, S, K, NT = 16, 128, 49, 5
P = 128
KT = EMBED // P  # 4 k-tiles over the 512 contraction dim

NCORES = 8
BG, VG = 2, 4          # batch groups x vocab groups
BPC = B // BG          # 8 batches per core
NB = BPC + 1           # + sentinel (global batch 0) block
TOK = BPC * S          # 1024 "real" token columns
TOKS = NB * S          # 1152 columns incl sentinel block
VPC = VOCAB // VG      # 7500 vocab columns per core
NCHUNK = 500           # psum n-chunk / W piece width
NPIECE = VPC // NCHUNK  # 15

# chunks of the (b,s) column axis for the r/s matmuls
_CHUNKS = [(0, 512), (512, 512), (1024, S)]

TRACE = False
LAST_EXEC_NS = None
_CACHE = {}


def _ap(base, dims):
    return bass.AP(base.tensor, base.offset, dims)


def _bld(gama_branch: bool, mlp_bias: bool):
    nc = bacc.Bacc(None, target_bir_lowering=False, debug=False)

    def din(name, shape):
        return nc.dram_tensor(name, shape, F32, kind="ExternalInput")

    hT = din("hT", [NB, EMBED, S])
    Vn = din("Vn", [NB, K, HIDDEN])
    VT = din("VT", [HIDDEN, NB * K])
    Tn = din("Tn", [NB, NT, EMBED])
    TT = din("TT", [EMBED, NB * NT])
    Wpk = din("Wpk", [HIDDEN, K + NT])
    WZVT = din("WZVT", [HIDDEN, K])
    WQTT = din("WQTT", [EMBED, NT])
    WsqT = din("WsqT", [EMBED, EMBED])
    WshT = din("WshT", [HIDDEN, EMBED])
    WszT = din("WszT", [HIDDEN, EMBED])
    WSsT = din("WSsT", [EMBED, NT])
    WSrT = din("WSrT", [EMBED, NT])
    Wbqc = din("Wbqc", [NT, 1])
    azr = din("azr", [1, K])
    bqr = din("bqr", [1, NT])
    bZd = din("bZd", [K, 1])
    bQd = din("bQd", [NT, 1])
    bQhd = din("bQhd", [NT, 1])
    bSsd = din("bSsd", [NT, 1])
    bSrd = din("bSrd", [NT, 1])
    bsqsh = din("bsqsh", [KT, P])
    bszd = din("bszd", [KT, P])
    WmT = din("WmT", [EMBED, VPC])
    bmr = din("bmr", [1, VPC])
    out = nc.dram_tensor("out", [BPC, NPIECE, S, NCHUNK], F32, kind="ExternalOutput")

    def r4(dram_ap):
        # [512, N] dram view -> [128, 4, N] (partition = h%128, kt = h//128)
        return dram_ap.rearrange("(kt pp) n -> pp kt n", pp=P)

    with tile.TileContext(nc) as tc, ExitStack() as ctx:
        pool = lambda name, bufs: ctx.enter_context(tc.tile_pool(name=name, bufs=bufs))
        const = pool("const", 1)
        hpool = pool("hpool", 1)     # hT_all, later reused for cT
        vtp = pool("vtp", 1)
        bigp = pool("bigp", 3)       # zT / qT / rT / sT rotate through 3 slots
        stp = pool("stp", 1)         # sT stays alive until cT done
        cntp = pool("cntp", 2)
        fvp = pool("fvp", 1)
        smp = pool("smp", 3)
        vstr = pool("vstr", 2)
        tstr = pool("tstr", 2)
        wfe = pool("wfe", 3)         # front-end big weights (Wsq/Wsh/Wsz)
        wmp = pool("wmp", 2)         # streamed W_mlp pieces
        evp = pool("evp", 3)
        bmp = pool("bmp", 2)
        psf = ctx.enter_context(tc.tile_pool(name="psf", bufs=3, space="PSUM"))
        pst = ctx.enter_context(tc.tile_pool(name="pst", bufs=2, space="PSUM"))
        psm = ctx.enter_context(tc.tile_pool(name="psm", bufs=3, space="PSUM"))

        # ---- constants / small weights ----
        ident = const.tile([P, P], F32)
        make_identity(nc, ident[:])
        Wpk_sb = const.tile([P, KT, K + NT], F32)
        nc.sync.dma_start(Wpk_sb[:], r4(Wpk[:]))
        WZVT_sb = const.tile([P, KT, K], F32)
        nc.sync.dma_start(WZVT_sb[:], r4(WZVT[:]))
        WQTT_sb = const.tile([P, KT, NT], F32)
        nc.sync.dma_start(WQTT_sb[:], r4(WQTT[:]))
        bZ_sb = const.tile([K, 1], F32)
        nc.sync.dma_start(bZ_sb[:], bZd[:])
        bQ_sb = const.tile([NT, 1], F32)
        nc.sync.dma_start(bQ_sb[:], bQd[:])
        bsqsh_sb = const.tile([P, KT], F32)
        nc.sync.dma_start(bsqsh_sb[:], _ap(bsqsh[:], [[1, P], [P, KT]]))
        if gama_branch:
            WSsT_sb = const.tile([P, KT, NT], F32)
            nc.sync.dma_start(WSsT_sb[:], r4(WSsT[:]))
            WSrT_sb = const.tile([P, KT, NT], F32)
            nc.sync.dma_start(WSrT_sb[:], r4(WSrT[:]))
            Wbq_sb = const.tile([NT, 1], F32)
            nc.sync.dma_start(Wbq_sb[:], Wbqc[:])
            bQh_sb = const.tile([NT, 1], F32)
            nc.sync.dma_start(bQh_sb[:], bQhd[:])
            bSs_sb = const.tile([NT, 1], F32)
            nc.sync.dma_start(bSs_sb[:], bSsd[:])
            bSr_sb = const.tile([NT, 1], F32)
            nc.sync.dma_start(bSr_sb[:], bSrd[:])
            bsz_sb = const.tile([P, KT], F32)
            nc.sync.dma_start(bsz_sb[:], _ap(bszd[:], [[1, P], [P, KT]]))
        ones_sb = const.tile([1, P], F32)
        nc.vector.memset(ones_sb[:], 1.0)
        # row constants broadcast across partitions (DRAM-source bcast DMA)
        az_bc = const.tile([P, K], F32)
        ab = azr[:]
        nc.sync.dma_start(az_bc[:], _ap(ab, [[0, P], [1, K]]))
        bq_bc = const.tile([P, NT], F32)
        bb = bqr[:]
        nc.sync.dma_start(bq_bc[:], _ap(bb, [[0, P], [1, NT]]))

        WsqT_sb = wfe.tile([P, KT, EMBED], F32, tag="wfe")
        nc.sync.dma_start(WsqT_sb[:], r4(WsqT[:]))
        WshT_sb = wfe.tile([P, KT, EMBED], F32, tag="wfe")
        nc.sync.dma_start(WshT_sb[:], r4(WshT[:]))
        if gama_branch:
            WszT_sb = wfe.tile([P, KT, EMBED], F32, tag="wfe")
            nc.sync.dma_start(WszT_sb[:], r4(WszT[:]))

        # ---- activations in ----
        hT_sb = hpool.tile([P, KT, TOKS], F32, tag="hc")
        for bi in range(NB):
            nc.sync.dma_start(hT_sb[:, :, bi * S:(bi + 1) * S], r4(hT[bi]))
        VT_sb = vtp.tile([P, KT, NB * K], F32)
        nc.sync.dma_start(VT_sb[:], r4(VT[:]))
        TT_sb = const.tile([P, KT, NB * NT], F32)
        nc.sync.dma_start(TT_sb[:], r4(TT[:]))

        # ---- VW' = W_ZV^T.T @ V^T + (b_ZV+b_Zh), all batches:  [j=49, (b,k)=441]
        if gama_branch:
            ps_vw = psf.tile([K, NB * K], F32, tag="psf")
            for kt in range(KT):
                nc.tensor.matmul(ps_vw[:], WZVT_sb[:, kt, :], VT_sb[:, kt, :],
                                 start=(kt == 0), stop=(kt == KT - 1))
            vwp = const.tile([K, NB * K], F32)
            nc.scalar.activation(vwp[:], ps_vw[:], AF.Identity, bias=bZ_sb[:])

        # qT' = W_QT^T.T @ T^T + (b_QT+b_Qh): [j=5, (b,t)=45]
        ps_qt = psf.tile([NT, NB * NT], F32, tag="psf")
        for kt in range(KT):
            nc.tensor.matmul(ps_qt[:], WQTT_sb[:, kt, :], TT_sb[:, kt, :],
                             start=(kt == 0), stop=(kt == KT - 1))
        qtp = const.tile([NT, NB * NT], F32)
        nc.scalar.activation(qtp[:], ps_qt[:], AF.Identity, bias=bQ_sb[:])

        # ---- per-batch front-end ----
        zT_sb = (bigp.tile([P, KT, TOKS], F32, tag="big", name="zT_sb")
                 if gama_branch else None)
        qT_sb = bigp.tile([P, KT, TOKS], F32, tag="big")

        for bi in range(NB):
            cs = bi * S  # column start for this batch block
            # h @ [W_Zh^T | W_Qh^T]  -> [tokens, 54]
            ps_h = psf.tile([P, K + NT], F32, tag="psf")
            for kt in range(KT):
                nc.tensor.matmul(ps_h[:], hT_sb[:, kt, cs:cs + S], Wpk_sb[:, kt, :],
                                 start=(kt == 0), stop=(kt == KT - 1))
            hwq = smp.tile([P, K + NT], F32, tag="hwq")
            nc.scalar.activation(hwq[:], ps_h[:], AF.Copy)

            if gama_branch:
                # content_V[t, k, j] = hW[t, j] + VW'[bi][k, j]; tanh; .az; sum_j
                # VW' row is flattened to a single partition, broadcast across
                # partitions via a K=1 ones-matmul (PSUM), then the hW term is
                # added during the DVE eviction.
                fV = fvp.tile([1, K * K], F32, tag="fV", name="fV")
                rowv = fV[:]
                nc.sync.dma_start(
                    _ap(rowv, [rowv.ap[0], [K, K], [1, K]]),  # (j outer, k runs)
                    vwp[:, bi * K:(bi + 1) * K])
                # cont is laid out [p, j, k] (flat row is j-outer)
                cont = cntp.tile([P, K, K], F32, tag="cont")
                hb = hwq[:]
                for jg0, ng in ((0, 10), (10, 10), (20, 10), (30, 10), (40, 9)):
                    ps_c = psf.tile([P, 490], F32, tag="psf", name="ps_c")
                    cw = ng * K
                    nc.tensor.matmul(ps_c[:, :cw], ones_sb[:],
                                     fV[0:1, jg0 * K:jg0 * K + cw],
                                     start=True, stop=True)
                    hW_b = bass.AP(hb.tensor, hb.offset + jg0,
                                   [hb.ap[0], [1, ng], [0, K]])
                    nc.vector.tensor_tensor(
                        cont[:, jg0:jg0 + ng, :],
                        ps_c[:, :cw].rearrange("p (a b) -> p a b", b=K),
                        hW_b, op=ALU.add)
                nc.scalar.activation(cont[:], cont[:], AF.Tanh)
                ab = az_bc[:]
                az_b = _ap(ab, [ab.ap[0], [1, K], [0, K]])
                nc.vector.tensor_tensor(cont[:], cont[:], az_b, op=ALU.mult)
                vis = smp.tile([P, K], F32, tag="vis")
                cb = cont[:]
                cont_kj = _ap(cb, [cb.ap[0], [1, K], [K, K]])  # [p, k, j] view
                nc.vector.tensor_reduce(vis[:], cont_kj, axis=AX.X, op=ALU.add)
                # softmax over k
                nmx = smp.tile([P, 1], F32, tag="nmx")
                nc.vector.tensor_reduce(nmx[:], vis[:], axis=AX.X, op=ALU.max,
                                        negate=True)
                sume = smp.tile([P, 1], F32, tag="sume")
                nc.scalar.activation(vis[:], vis[:], AF.Exp, bias=nmx[:],
                                     accum_out=sume[:])
                rcp = smp.tile([P, 1], F32, tag="rcp")
                nc.vector.reciprocal(rcp[:], sume[:])
                nc.vector.tensor_scalar_mul(vis[:], vis[:], rcp[:])
                # alpha^T via PE transpose
                ps_at = pst.tile([K, P], F32, tag="pst")
                nc.tensor.transpose(ps_at[:], vis[:], ident[:])
                aT = smp.tile([K, P], F32, tag="aT")
                nc.scalar.activation(aT[:], ps_at[:], AF.Copy)
                # z^T = V^T @ alpha^T
                V_b = vstr.tile([K, HIDDEN], F32, tag="vb")
                nc.sync.dma_start(V_b[:], Vn[bi])
                for mt in range(KT):
                    ps_z = pst.tile([P, S], F32, tag="pst")
                    nc.tensor.matmul(ps_z[:], V_b[:, mt * P:(mt + 1) * P], aT[:],
                                     start=True, stop=True)
                    nc.scalar.activation(zT_sb[:, mt, cs:cs + S], ps_z[:], AF.Copy)

            # topic attention
            fT = smp.tile([1, NT * NT], F32, tag="fT", name="fT")
            rowt = fT[:]
            nc.sync.dma_start(
                _ap(rowt, [rowt.ap[0], [NT, NT], [1, NT]]),
                qtp[:, bi * NT:(bi + 1) * NT])
            # cont_t is laid out [p, j, t] (flat row is j-outer)
            cont_t = smp.tile([P, NT, NT], F32, tag="cont_t")
            hb = hwq[:]
            ps_ct = pst.tile([P, NT * NT], F32, tag="pst", name="ps_ct")
            nc.tensor.matmul(ps_ct[:], ones_sb[:], fT[0:1, :],
                             start=True, stop=True)
            hQ_b = bass.AP(hb.tensor, hb.offset + K, [hb.ap[0], [1, NT], [0, NT]])
            nc.vector.tensor_tensor(
                cont_t[:], ps_ct[:].rearrange("p (a b) -> p a b", b=NT),
                hQ_b, op=ALU.add)
            nc.scalar.activation(cont_t[:], cont_t[:], AF.Tanh)
            bb = bq_bc[:]
            bq_b = _ap(bb, [bb.ap[0], [1, NT], [0, NT]])
            nc.vector.tensor_tensor(cont_t[:], cont_t[:], bq_b, op=ALU.mult)
            top = smp.tile([P, NT], F32, tag="top")
            ctb = cont_t[:]
            cont_tj = _ap(ctb, [ctb.ap[0], [1, NT], [NT, NT]])  # [p, t, j] view
            nc.vector.tensor_reduce(top[:], cont_tj, axis=AX.X, op=ALU.add)
            nmxt = smp.tile([P, 1], F32, tag="nmxt")
            nc.vector.tensor_reduce(nmxt[:], top[:], axis=AX.X, op=ALU.max,
                                    negate=True)
            sumt = smp.tile([P, 1], F32, tag="sumt")
            nc.scalar.activation(top[:], top[:], AF.Exp, bias=nmxt[:],
                                 accum_out=sumt[:])
            rct = smp.tile([P, 1], F32, tag="rct")
            nc.vector.reciprocal(rct[:], sumt[:])
            nc.vector.tensor_scalar_mul(top[:], top[:], rct[:])
            ps_bt = pst.tile([NT, P], F32, tag="pst")
            nc.tensor.transpose(ps_bt[:], top[:], ident[:])
            bT = smp.tile([NT, P], F32, tag="bT")
            nc.scalar.activation(bT[:], ps_bt[:], AF.Copy)
            T_b = tstr.tile([NT, EMBED], F32, tag="tb")
            nc.sync.dma_start(T_b[:], Tn[bi])
            for mt in range(KT):
                ps_q = pst.tile([P, S], F32, tag="pst")
                nc.tensor.matmul(ps_q[:], T_b[:, mt * P:(mt + 1) * P], bT[:],
                                 start=True, stop=True)
                nc.scalar.activation(qT_sb[:, mt, cs:cs + S], ps_q[:], AF.Copy)

        # ---- r^T = tanh(W_sz^T.T @ z^T + b_sz) ----
        if gama_branch:
            rT_sb = bigp.tile([P, KT, TOKS], F32, tag="big")
            for mt in range(KT):
                for c0, cw in _CHUNKS:
                    ps_r = psf.tile([P, 512], F32, tag="psf")
                    for kt in range(KT):
                        nc.tensor.matmul(ps_r[:, :cw],
                                         WszT_sb[:, kt, mt * P:(mt + 1) * P],
                                         zT_sb[:, kt, c0:c0 + cw],
                                         start=(kt == 0), stop=(kt == KT - 1))
                    nc.scalar.activation(rT_sb[:, mt, c0:c0 + cw], ps_r[:, :cw],
                                         AF.Tanh, bias=bsz_sb[:, mt:mt + 1])

        # ---- s^T = tanh(W_sq^T.T @ q^T + W_sh^T.T @ h^T + b) ----
        sT_sb = stp.tile([P, KT, TOKS], F32)
        for mt in range(KT):
            for c0, cw in _CHUNKS:
                ps_s = psf.tile([P, 512], F32, tag="psf")
                for kt in range(KT):
                    nc.tensor.matmul(ps_s[:, :cw],
                                     WsqT_sb[:, kt, mt * P:(mt + 1) * P],
                                     qT_sb[:, kt, c0:c0 + cw],
                                     start=(kt == 0), stop=False)
                for kt in range(KT):
                    nc.tensor.matmul(ps_s[:, :cw],
                                     WshT_sb[:, kt, mt * P:(mt + 1) * P],
                                     hT_sb[:, kt, c0:c0 + cw],
                                     start=False, stop=(kt == KT - 1))
                nc.scalar.activation(sT_sb[:, mt, c0:c0 + cw], ps_s[:, :cw],
                                     AF.Tanh, bias=bsqsh_sb[:, mt:mt + 1])

        # ---- sentinel gate gama from the 9th (global batch 0) block ----
        if gama_branch:
            sc0 = BPC * S
            ps_u = pst.tile([NT, S], F32, tag="pst")
            for kt in range(KT):
                nc.tensor.matmul(ps_u[:], WSsT_sb[:, kt, :],
                                 sT_sb[:, kt, sc0:sc0 + S],
                                 start=(kt == 0), stop=(kt == KT - 1))
            us = smp.tile([NT, S], F32, tag="us")
            nc.scalar.activation(us[:], ps_u[:], AF.Identity, bias=bSs_sb[:])
            ps_u2 = pst.tile([NT, S], F32, tag="pst")
            for kt in range(KT):
                nc.tensor.matmul(ps_u2[:], WSrT_sb[:, kt, :],
                                 rT_sb[:, kt, sc0:sc0 + S],
                                 start=(kt == 0), stop=(kt == KT - 1))
            ur = smp.tile([NT, S], F32, tag="ur")
            nc.scalar.activation(ur[:], ps_u2[:], AF.Identity, bias=bSr_sb[:])
            ps_hq = pst.tile([NT, S], F32, tag="pst")
            for kt in range(KT):
                nc.tensor.matmul(ps_hq[:], Wpk_sb[:, kt, K:K + NT],
                                 hT_sb[:, kt, sc0:sc0 + S],
                                 start=(kt == 0), stop=(kt == KT - 1))
            hq0 = smp.tile([NT, S], F32, tag="hq0")
            nc.scalar.activation(hq0[:], ps_hq[:], AF.Identity, bias=bQh_sb[:])
            nc.vector.tensor_add(us[:], us[:], hq0[:])
            nc.scalar.activation(us[:], us[:], AF.Tanh)
            nc.vector.tensor_add(ur[:], ur[:], hq0[:])
            nc.scalar.activation(ur[:], ur[:], AF.Tanh)
            ps_ss = pst.tile([1, S], F32, tag="pst")
            nc.tensor.matmul(ps_ss[:], Wbq_sb[:], us[:], start=True, stop=True)
            ss = smp.tile([1, S], F32, tag="ss")
            nc.scalar.activation(ss[:], ps_ss[:], AF.Copy)
            ps_sr = pst.tile([1, S], F32, tag="pst")
            nc.tensor.matmul(ps_sr[:], Wbq_sb[:], ur[:], start=True, stop=True)
            dsc = smp.tile([1, S], F32, tag="dsc")
            nc.vector.tensor_sub(dsc[:], ss[:], ps_sr[:])
            gama = smp.tile([1, S], F32, tag="gama")
            nc.scalar.activation(gama[:], dsc[:], AF.Sigmoid)
            # broadcast gama across partitions via ones-matmul
            ps_g = pst.tile([P, S], F32, tag="pst", name="ps_g")
            nc.tensor.matmul(ps_g[:], ones_sb[:], gama[:], start=True, stop=True)
            g_sb = smp.tile([P, S], F32, tag="g_sb")
            nc.scalar.activation(g_sb[:], ps_g[:], AF.Copy)

            # c^T = r^T + gama * (s^T - r^T)   on the 8 real batch blocks
            cT_sb = hpool.tile([P, KT, TOKS], F32, tag="hc")
            gb = g_sb[:]
            g_b = _ap(gb, [gb.ap[0], [0, BPC], [1, S]])
            for mt in range(KT):
                sv = sT_sb[:, mt, 0:TOK].rearrange("p (b s) -> p b s", b=BPC)
                rv = rT_sb[:, mt, 0:TOK].rearrange("p (b s) -> p b s", b=BPC)
                cv = cT_sb[:, mt, 0:TOK].rearrange("p (b s) -> p b s", b=BPC)
                nc.vector.tensor_sub(cv, sv, rv)
                nc.vector.tensor_tensor(cv, cv, g_b, op=ALU.mult)
                nc.vector.tensor_add(cv, cv, rv)
        else:
            cT_sb = sT_sb

        # ---- MLP: scores = c^T.T @ W_mlp^T (+ b_mlp) ----
        for p in range(NPIECE):
            Wp = wmp.tile([P, KT, NCHUNK], F32, tag="wp")
            nc.sync.dma_start(
                Wp[:], r4(WmT[:])[:, :, p * NCHUNK:(p + 1) * NCHUNK])
            if mlp_bias:
                bm = bmp.tile([1, NCHUNK], F32, tag="bm")
                nc.sync.dma_start(bm[:], bmr[:, p * NCHUNK:(p + 1) * NCHUNK])
            for m in range(BPC):
                ps_o = psm.tile([P, NCHUNK], F32, tag="mm")
                if mlp_bias:
                    nc.tensor.matmul(ps_o[:], ones_sb[:], bm[:],
                                     start=True, stop=False)
                for kt in range(KT):
                    nc.tensor.matmul(ps_o[:], cT_sb[:, kt, m * S:(m + 1) * S],
                                     Wp[:, kt, :],
                                     start=(kt == 0 and not mlp_bias),
                                     stop=(kt == KT - 1))
                ob = evp.tile([P, NCHUNK], F32, tag="ev")
                nc.scalar.activation(ob[:], ps_o[:], AF.Copy)
                nc.sync.dma_start(out[m, p], ob[:])

    nc.compile()
    return nc


def _get(gama_branch, mlp_bias):
    key = (bool(gama_branch), bool(mlp_bias))
    if key not in _CACHE:
        _CACHE[key] = _bld(*key)
    return _CACHE[key]


def prepare(epoch, h_t, V, T,
            W_ZV, b_ZV, W_Zh, b_Zh, W_az, b_az,
            W_QT, b_QT, W_Qh, b_Qh, W_bq, b_bq,
            W_sq, b_sq, W_sh, b_sh, W_Ss, b_Ss,
            W_Sr, b_Sr, W_sz, b_sz, W_mlp, b_mlp):
    """Host-side prep: returns (nc, in_maps) for run_bass_kernel_spmd."""
    f = lambda x: np.ascontiguousarray(np.asarray(x, dtype=np.float32))
    h_t, V, T = f(h_t), f(V), f(T)
    W_ZV, b_ZV, W_Zh, b_Zh = f(W_ZV), f(b_ZV), f(W_Zh), f(b_Zh)
    W_az, W_QT, b_QT, W_Qh, b_Qh = f(W_az), f(W_QT), f(b_QT), f(W_Qh), f(b_Qh)
    W_bq, W_sq, b_sq, W_sh, b_sh = f(W_bq), f(W_sq), f(b_sq), f(W_sh), f(b_sh)
    W_Ss, b_Ss, W_Sr, b_Sr = f(W_Ss), f(b_Ss), f(W_Sr), f(b_Sr)
    W_sz, b_sz, W_mlp, b_mlp = f(W_sz), f(b_sz), f(W_mlp), f(b_mlp)

    gama_branch = int(np.asarray(epoch)) > 20
    mlp_bias = bool(np.any(b_mlp))
    nc = _get(gama_branch, mlp_bias)

    hTt = np.ascontiguousarray(h_t.transpose(0, 2, 1))      # [16, 512, 128]
    shared = {
        "Wpk": np.ascontiguousarray(
            np.concatenate([W_Zh.T, W_Qh.T], axis=1)),
        "WZVT": np.ascontiguousarray(W_ZV.T),
        "WQTT": np.ascontiguousarray(W_QT.T),
        "WsqT": np.ascontiguousarray(W_sq.T),
        "WshT": np.ascontiguousarray(W_sh.T),
        "WszT": np.ascontiguousarray(W_sz.T),
        "WSsT": np.ascontiguousarray(W_Ss.T),
        "WSrT": np.ascontiguousarray(W_Sr.T),
        "Wbqc": np.ascontiguousarray(W_bq.T),
        "azr": W_az.reshape(1, K),
        "bqr": W_bq.reshape(1, NT),
        "bZd": (b_ZV + b_Zh).reshape(K, 1),
        "bQd": (b_QT + b_Qh).reshape(NT, 1),
        "bQhd": b_Qh.reshape(NT, 1),
        "bSsd": b_Ss.reshape(NT, 1),
        "bSrd": b_Sr.reshape(NT, 1),
        "bsqsh": (b_sq + b_sh).reshape(KT, P),
        "bszd": b_sz.reshape(KT, P),
    }
    WmTfull = np.ascontiguousarray(W_mlp.T)                 # [512, 30000]

    in_maps = []
    for c in range(NCORES):
        bg, vg = divmod(c, VG)
        bs = slice(bg * BPC, (bg + 1) * BPC)
        vs = slice(vg * VPC, (vg + 1) * VPC)
        h9 = np.concatenate([hTt[bs], hTt[0:1]], axis=0)    # [9, 512, 128]
        V9 = np.concatenate([V[bs], V[0:1]], axis=0)        # [9, 49, 512]
        T9 = np.concatenate([T[bs], T[0:1]], axis=0)        # [9, 5, 512]
        m = dict(shared)
        m["hT"] = np.ascontiguousarray(h9)
        m["Vn"] = np.ascontiguousarray(V9)
        m["VT"] = np.ascontiguousarray(
            V9.transpose(2, 0, 1).reshape(HIDDEN, NB * K))
        m["Tn"] = np.ascontiguousarray(T9)
        m["TT"] = np.ascontiguousarray(
            T9.transpose(2, 0, 1).reshape(EMBED, NB * NT))
        m["WmT"] = np.ascontiguousarray(WmTfull[:, vs])
        m["bmr"] = np.ascontiguousarray(b_mlp[None, vs])
        in_maps.append(m)
    return nc, in_maps


def assemble(results):
    full = np.empty((B, S, VOCAB), np.float32)
    for c in range(NCORES):
        bg, vg = divmod(c, VG)
        arr = results[c]["out"]                             # [8, 15, 128, 500]
        full[bg * BPC:(bg + 1) * BPC, :, vg * VPC:(vg + 1) * VPC] = (
            arr.transpose(0, 2, 1, 3).reshape(BPC, S, VPC))
    return full


def kernel(**inputs):
    global LAST_EXEC_NS
    nc, in_maps = prepare(**inputs)
    res = run_bass_kernel_spmd(nc, in_maps, core_ids=list(range(NCORES)),
                               trace=TRACE)
    LAST_EXEC_NS = res.exec_time_ns
    return assemble(res.results)
